# revision 8
# baseline (speedup 1.0000x reference)
"""Bahdanau attention on 8 trn2 NeuronCores, data-parallel over batch.

Reference computation (per batch b):
    proj_f = features @ W1 + b1            # [L, U]
    proj_h = hidden @ W2 + b2              # [U]
    score  = tanh(proj_f + proj_h) @ V     # [L]  (+bV, irrelevant to softmax)
    w      = softmax(score)                # [L]
    ctx    = sum_l w[l] * features[l, :]   # [D]
Outputs: (ctx [B, D], w [B, L, 1])

Shapes: B=64, L=D=H=U=1024. 8 cores x 8 batches each.

Per-core kernel layout:
  - proj computed transposed: projT[u, l] = sum_d W1[d, u] * fT[d, l], with
    W1 chunks as stationary lhsT and host-transposed features (fT) streamed.
  - tanh(proj + bias) fused on the scalar engine with per-partition bias
    bias[u] = proj_h[b, u] + b1[u] + b2[u] (biases folded into the proj_h
    matmul via K=1 ones-row matmuls).
  - score = V^T @ tanh accumulated in PSUM; per-batch softmax on one
    partition; context via w-as-stationary matmuls over natural features.
  - Matmuls use float32r (TF32-like fast fp32 path, fp32 accumulate);
    inputs are rounded to float32r by DVE copies as the hardware requires.
"""

import numpy as np

import concourse.bass as bass
import concourse.mybir as mybir
import concourse.tile as tile
from concourse import bacc
from concourse.bass_utils import run_bass_kernel_spmd

P = 128
L = 1024
D = 1024
H = 1024
U = 1024
B = 64
N_CORES = 8
B_SHARD = B // N_CORES

F32 = mybir.dt.float32
F32R = mybir.dt.float32r


def build_nc(b_shard: int = B_SHARD):
    KD = D // P      # 8 contraction chunks over d
    KH = H // P      # 8 contraction chunks over h
    CU = U // P      # 8 u chunks
    CL = L // P      # 8 l chunks
    NH = 512         # moving-dim tile (fp32 max)

    nc = bacc.Bacc(None)

    featT = nc.declare_dram_parameter("featT", [b_shard, D, L], F32, isOutput=False)
    feat = nc.declare_dram_parameter("feat", [b_shard, L, D], F32, isOutput=False)
    hiddenT = nc.declare_dram_parameter("hiddenT", [H, b_shard], F32, isOutput=False)
    W1 = nc.declare_dram_parameter("W1", [D, U], F32, isOutput=False)
    W2 = nc.declare_dram_parameter("W2", [H, U], F32, isOutput=False)
    V = nc.declare_dram_parameter("V", [U, 1], F32, isOutput=False)
    b1 = nc.declare_dram_parameter("b1", [U], F32, isOutput=False)
    b2 = nc.declare_dram_parameter("b2", [U], F32, isOutput=False)
    ctx_out = nc.declare_dram_parameter("ctx", [b_shard, D], F32, isOutput=True)
    attn_out = nc.declare_dram_parameter("attn", [b_shard, L], F32, isOutput=True)

    with tile.TileContext(nc) as tc:
        with (
            tc.tile_pool(name="const", bufs=1) as const,
            tc.tile_pool(name="stage", bufs=3) as stage,
            tc.tile_pool(name="ftr", bufs=10) as ftr_pool,
            tc.tile_pool(name="tanh", bufs=4) as tanh_pool,
            tc.tile_pool(name="fr", bufs=10) as fr_pool,
            tc.tile_pool(name="row", bufs=2) as row_pool,
            tc.tile_pool(name="dram", bufs=2, space="DRAM") as dram,
            tc.tile_pool(name="pp", bufs=2, space="PSUM") as pp,
            tc.tile_pool(name="psm", bufs=4, space="PSUM") as psm,
        ):
            # ---- constants / weights resident in SBUF ----
            w1r = const.tile([P, KD, U], F32R, tag="w1r")
            for k in range(KD):
                w1_st = stage.tile([P, U], F32, tag="st")
                nc.sync.dma_start(w1_st[:], W1[k * P : (k + 1) * P, :])
                nc.vector.tensor_copy(w1r[:, k], w1_st[:])

            v_sb = const.tile([P, CU], F32, tag="vsb")
            nc.sync.dma_start(v_sb[:], V[:].rearrange("(c p) o -> p (c o)", p=P))
            v_r = const.tile([P, CU], F32R, tag="vr")
            nc.vector.tensor_copy(v_r[:], v_sb[:])

            ht_sb = const.tile([P, KH, b_shard], F32, tag="ht")
            nc.sync.dma_start(ht_sb[:], hiddenT[:].rearrange("(k p) b -> p k b", p=P))
            ones_sb = const.tile([1, b_shard], F32, tag="ones")
            nc.vector.memset(ones_sb[:], 1.0)
            b1_sb = const.tile([1, U], F32, tag="b1")
            nc.sync.dma_start(b1_sb[:], b1[:].unsqueeze(0))
            b2_sb = const.tile([1, U], F32, tag="b2")
            nc.sync.dma_start(b2_sb[:], b2[:].unsqueeze(0))

            # ---- proj_h + biases: bias[b, u] = hidden@W2 + b2 + b1 ----
            ph_ps = [psm.tile([b_shard, NH], F32, tag="small", name=f"ph_ps{h}") for h in range(2)]
            for k in range(KH):
                w2_st = stage.tile([P, U], F32, tag="st")
                nc.sync.dma_start(w2_st[:], W2[k * P : (k + 1) * P, :])
                for h in range(2):
                    nc.tensor.matmul(
                        ph_ps[h][:, :],
                        ht_sb[:, k],
                        w2_st[:, h * NH : (h + 1) * NH],
                        start=(k == 0),
                        stop=False,
                    )
            for h in range(2):
                nc.tensor.matmul(
                    ph_ps[h][:, :], ones_sb[:], b1_sb[:, h * NH : (h + 1) * NH],
                    start=False, stop=False,
                )
                nc.tensor.matmul(
                    ph_ps[h][:, :], ones_sb[:], b2_sb[:, h * NH : (h + 1) * NH],
                    start=False, stop=True,
                )
            bias_sb = const.tile([b_shard, U], F32, tag="bias")
            for h in range(2):
                nc.vector.tensor_copy(bias_sb[:, h * NH : (h + 1) * NH], ph_ps[h][:, :])
            # transpose to [u-in-chunk, (chunk, b)] for per-partition ACT bias
            biasT = const.tile([P, CU * b_shard], F32, tag="biasT")
            b_scr = dram.tile([b_shard, U], F32, tag="bscr")
            nc.sync.dma_start(b_scr[:], bias_sb[:])
            with nc.allow_non_contiguous_dma(reason="tiny one-time bias transpose"):
                for c in range(CU):
                    nc.sync.dma_start(
                        biasT[:, c * b_shard : (c + 1) * b_shard],
                        b_scr[:, c * P : (c + 1) * P].rearrange("b l -> l b"),
                    )

            ctx_row = [None] * b_shard
            # ---- main per-batch loop ----
            for b in range(b_shard):
                # stream + round fT chunks for this batch
                ftr = []
                for k in range(KD):
                    ft_st = stage.tile([P, L], F32, tag="st")
                    nc.sync.dma_start(ft_st[:], featT[b, k * P : (k + 1) * P, :])
                    t = ftr_pool.tile([P, L], F32R, tag="ftr")
                    nc.vector.tensor_copy(t[:], ft_st[:])
                    ftr.append(t)
                # natural-layout features for the context phase
                f_r = []
                for c in range(CL):
                    f_st = stage.tile([P, D], F32, tag="st")
                    nc.sync.dma_start(f_st[:], feat[b, c * P : (c + 1) * P, :])
                    t = fr_pool.tile([P, D], F32R, tag="fr")
                    nc.vector.tensor_copy(t[:], f_st[:])
                    f_r.append(t)

                score_ps = [psm.tile([1, NH], F32, tag="small", name=f"score_ps{h}") for h in range(2)]
                for cu in range(CU):
                    ps = pp.tile([P, L], F32, tag="proj")
                    for k in range(KD):
                        lhsT = w1r[:, k, cu * P : (cu + 1) * P]
                        nc.tensor.matmul(
                            ps[:, 0:NH], lhsT, ftr[k][:, 0:NH],
                            start=(k == 0), stop=(k == KD - 1),
                        )
                        nc.tensor.matmul(
                            ps[:, NH:L], lhsT, ftr[k][:, NH:L],
                            start=(k == 0), stop=(k == KD - 1),
                        )
                    th = tanh_pool.tile([P, L], F32R, tag="tanh")
                    nc.scalar.activation(
                        th[:], ps[:],
                        mybir.ActivationFunctionType.Tanh,
                        bias=biasT[:, cu * b_shard + b : cu * b_shard + b + 1],
                    )
                    for h in range(2):
                        nc.tensor.matmul(
                            score_ps[h][:, :],
                            v_r[:, cu : cu + 1],
                            th[:, h * NH : (h + 1) * NH],
                            start=(cu == 0),
                            stop=(cu == CU - 1),
                        )

                # softmax over l on one partition
                scr = row_pool.tile([1, L], F32, tag="scr")
                for h in range(2):
                    nc.vector.tensor_copy(scr[:, h * NH : (h + 1) * NH], score_ps[h][:, :])
                nmax = row_pool.tile([1, 1], F32, tag="nmax")
                nc.vector.reduce_max(nmax[:], scr[:], axis=mybir.AxisListType.X, negate=True)
                ex = row_pool.tile([1, L], F32, tag="ex")
                nc.scalar.activation(
                    ex[:], scr[:], mybir.ActivationFunctionType.Exp, bias=nmax[:]
                )
                sm = row_pool.tile([1, 1], F32, tag="sm")
                nc.vector.reduce_sum(sm[:], ex[:], axis=mybir.AxisListType.X)
                rcp = row_pool.tile([1, 1], F32, tag="rcp")
                nc.vector.reciprocal(rcp[:], sm[:])
                w_row = row_pool.tile([1, L], F32, tag="wrow")
                nc.vector.tensor_scalar_mul(w_row[:], ex[:], rcp[:])
                nc.sync.dma_start(attn_out[b : b + 1, :], w_row[:])

                # transpose w to partitions, round, context matmuls
                wt = row_pool.tile([P, CL], F32, tag="wt")
                w_scr = dram.tile([1, L], F32, tag="wscr")
                nc.sync.dma_start(w_scr[:], w_row[:])
                with nc.allow_non_contiguous_dma(reason="tiny per-batch w transpose"):
                    nc.sync.dma_start(
                        wt[:], w_scr[0].rearrange("(c l) -> l c", l=P)
                    )
                wt_r = row_pool.tile([P, CL], F32R, tag="wtr")
                nc.vector.tensor_copy(wt_r[:], wt[:])

                ctx_ps = [psm.tile([1, NH], F32, tag="small", name=f"ctx_ps{h}") for h in range(2)]
                for c in range(CL):
                    for h in range(2):
                        nc.tensor.matmul(
                            ctx_ps[h][:, :],
                            wt_r[:, c : c + 1],
                            f_r[c][:, h * NH : (h + 1) * NH],
                            start=(c == 0),
                            stop=(c == CL - 1),
                        )
                crow = row_pool.tile([1, D], F32, tag="crow")
                for h in range(2):
                    nc.vector.tensor_copy(crow[:, h * NH : (h + 1) * NH], ctx_ps[h][:, :])
                nc.sync.dma_start(ctx_out[b : b + 1, :], crow[:])
                ctx_row[b] = crow

    nc.finalize()
    return nc


_NC_CACHE = {}


def _get_nc(b_shard: int = B_SHARD):
    if b_shard not in _NC_CACHE:
        _NC_CACHE[b_shard] = build_nc(b_shard)
    return _NC_CACHE[b_shard]


def make_in_maps(features, hidden, W1, b1, W2, b2, V, bV):
    features = np.ascontiguousarray(np.asarray(features, dtype=np.float32))
    hidden = np.asarray(hidden, dtype=np.float32)
    W1 = np.ascontiguousarray(np.asarray(W1, dtype=np.float32))
    W2 = np.ascontiguousarray(np.asarray(W2, dtype=np.float32))
    V = np.ascontiguousarray(np.asarray(V, dtype=np.float32).reshape(U, 1))
    b1 = np.ascontiguousarray(np.asarray(b1, dtype=np.float32))
    b2 = np.ascontiguousarray(np.asarray(b2, dtype=np.float32))

    in_maps = []
    for i in range(N_CORES):
        sl = slice(i * B_SHARD, (i + 1) * B_SHARD)
        f = features[sl]
        in_maps.append(
            {
                "featT": np.ascontiguousarray(f.transpose(0, 2, 1)),
                "feat": np.ascontiguousarray(f),
                "hiddenT": np.ascontiguousarray(hidden[sl].T),
                "W1": W1,
                "W2": W2,
                "V": V,
                "b1": b1,
                "b2": b2,
            }
        )
    return in_maps


def kernel(features, hidden, W1, b1, W2, b2, V, bV):
    nc = _get_nc()
    in_maps = make_in_maps(features, hidden, W1, b1, W2, b2, V, bV)
    res = run_bass_kernel_spmd(nc, in_maps, list(range(N_CORES)))
    ctx = np.concatenate([res.results[i]["ctx"] for i in range(N_CORES)], axis=0)
    attn = np.concatenate([res.results[i]["attn"] for i in range(N_CORES)], axis=0)
    return ctx.astype(np.float32), attn.reshape(B, L, 1).astype(np.float32)


# revision 12
# speedup vs baseline: 275.8858x; 275.8858x over previous
"""Bahdanau attention on 8 trn2 NeuronCores, data-parallel over batch.

Reference computation (per batch b):
    proj_f = features @ W1 + b1            # [L, U]
    proj_h = hidden @ W2 + b2              # [U]
    score  = tanh(proj_f + proj_h) @ V     # [L]  (+bV, irrelevant to softmax)
    w      = softmax(score)                # [L]
    ctx    = sum_l w[l] * features[l, :]   # [D]
Outputs: (ctx [B, D], w [B, L, 1])

Shapes: B=64, L=D=H=U=1024. 8 cores x 8 batches each.

Per-core kernel layout:
  - proj computed transposed: projT[u, l] = sum_d W1[d, u] * fT[d, l], with
    W1 chunks as stationary lhsT and host-transposed features (fT) streamed.
  - tanh(proj + bias) fused on the scalar engine with per-partition bias
    bias[u] = proj_h[b, u] + b1[u] + b2[u] (biases folded into the proj_h
    matmul via K=1 ones-row matmuls).
  - score = V^T @ tanh accumulated in PSUM; per-batch softmax on one
    partition; context via w-as-stationary matmuls over natural features.
  - Matmuls use float32r (TF32-like fast fp32 path, fp32 accumulate);
    inputs are rounded to float32r by DVE copies as the hardware requires.
"""

import numpy as np

import concourse.bass as bass
import concourse.mybir as mybir
import concourse.tile as tile
from concourse import bacc
from concourse.bass_utils import run_bass_kernel_spmd

P = 128
L = 1024
D = 1024
H = 1024
U = 1024
B = 64
N_CORES = 8
B_SHARD = B // N_CORES

F32 = mybir.dt.float32
F32R = mybir.dt.float32r


def build_nc(b_shard: int = B_SHARD):
    KD = D // P      # 8 contraction chunks over d
    KH = H // P      # 8 contraction chunks over h
    CU = U // P      # 8 u chunks
    CL = L // P      # 8 l chunks
    NH = 512         # moving-dim tile (fp32 max)

    nc = bacc.Bacc(None)

    featT = nc.declare_dram_parameter("featT", [b_shard, D, L], F32, isOutput=False)
    feat = nc.declare_dram_parameter("feat", [b_shard, L, D], F32, isOutput=False)
    hiddenT = nc.declare_dram_parameter("hiddenT", [H, b_shard], F32, isOutput=False)
    W1 = nc.declare_dram_parameter("W1", [D, U], F32, isOutput=False)
    W2 = nc.declare_dram_parameter("W2", [H, U], F32, isOutput=False)
    V = nc.declare_dram_parameter("V", [U, 1], F32, isOutput=False)
    b1 = nc.declare_dram_parameter("b1", [U], F32, isOutput=False)
    b2 = nc.declare_dram_parameter("b2", [U], F32, isOutput=False)
    ctx_out = nc.declare_dram_parameter("ctx", [b_shard, D], F32, isOutput=True)
    attn_out = nc.declare_dram_parameter("attn", [b_shard, L], F32, isOutput=True)

    with tile.TileContext(nc) as tc:
        with (
            tc.tile_pool(name="const", bufs=1) as const,
            tc.tile_pool(name="stage", bufs=stage_bufs) as stage,
            tc.tile_pool(name="ftr", bufs=ftr_bufs) as ftr_pool,
            tc.tile_pool(name="tanh", bufs=tanh_bufs) as tanh_pool,
            tc.tile_pool(name="fr", bufs=fr_bufs) as fr_pool,
            tc.tile_pool(name="row", bufs=2) as row_pool,
            tc.tile_pool(name="w2r", bufs=2) as w2r_pool,
            tc.tile_pool(name="dram", bufs=2, space="DRAM") as dram,
            tc.tile_pool(name="pp", bufs=pp_bufs, space="PSUM") as pp,
            tc.tile_pool(name="psm", bufs=psm_bufs, space="PSUM") as psm,
        ):
            # ---- constants / weights resident in SBUF ----
            w1r = const.tile([P, KD, U], F32R, tag="w1r")
            for k in range(KD):
                w1_st = stage.tile([P, U], F32, tag="st")
                nc.sync.dma_start(w1_st[:], W1[k * P : (k + 1) * P, :])
                nc.vector.tensor_copy(w1r[:, k], w1_st[:])

            v_sb = const.tile([P, CU], F32, tag="vsb")
            nc.sync.dma_start(v_sb[:], V[:].rearrange("(c p) o -> p (c o)", p=P))
            v_r = const.tile([P, CU], F32R, tag="vr")
            nc.vector.tensor_copy(v_r[:], v_sb[:])

            ht_sb = const.tile([P, KH, b_shard], F32, tag="ht")
            nc.sync.dma_start(ht_sb[:], hiddenT[:].rearrange("(k p) b -> p k b", p=P))
            ones_sb = const.tile([1, b_shard], F32, tag="ones")
            nc.vector.memset(ones_sb[:], 1.0)
            b1_sb = const.tile([1, U], F32, tag="b1")
            nc.sync.dma_start(b1_sb[:], b1[:].unsqueeze(0))
            b2_sb = const.tile([1, U], F32, tag="b2")
            nc.sync.dma_start(b2_sb[:], b2[:].unsqueeze(0))

            # ---- proj_h + biases: bias[b, u] = hidden@W2 + b2 + b1 ----
            ph_ps = [psm.tile([b_shard, NH], F32, tag="small", name=f"ph_ps{h}") for h in range(2)]
            for k in range(KH):
                w2_st = stage.tile([P, U], F32, tag="st")
                nc.sync.dma_start(w2_st[:], W2[k * P : (k + 1) * P, :])
                for h in range(2):
                    nc.tensor.matmul(
                        ph_ps[h][:, :],
                        ht_sb[:, k],
                        w2_st[:, h * NH : (h + 1) * NH],
                        start=(k == 0),
                        stop=False,
                    )
            for h in range(2):
                nc.tensor.matmul(
                    ph_ps[h][:, :], ones_sb[:], b1_r[:, h * NH : (h + 1) * NH],
                    start=False, stop=False,
                )
                nc.tensor.matmul(
                    ph_ps[h][:, :], ones_sb[:], b2_r[:, h * NH : (h + 1) * NH],
                    start=False, stop=True,
                )
            bias_sb = const.tile([b_shard, U], F32, tag="bias")
            for h in range(2):
                nc.vector.tensor_copy(bias_sb[:, h * NH : (h + 1) * NH], ph_ps[h][:, :])
            # transpose to [u-in-chunk, (chunk, b)] for per-partition ACT bias
            biasT = const.tile([P, CU * b_shard], F32, tag="biasT")
            b_scr = dram.tile([b_shard, U], F32, tag="bscr")
            nc.sync.dma_start(b_scr[:], bias_sb[:])
            with nc.allow_non_contiguous_dma(reason="tiny one-time bias transpose"):
                for c in range(CU):
                    nc.sync.dma_start(
                        biasT[:, c * b_shard : (c + 1) * b_shard],
                        b_scr[:, c * P : (c + 1) * P].rearrange("b l -> l b"),
                    )

            ctx_row = [None] * b_shard
            # ---- main per-batch loop ----
            for b in range(b_shard):
                # stream + round fT chunks for this batch
                ftr = []
                for k in range(KD):
                    ft_st = stage.tile([P, L], F32, tag="st")
                    nc.sync.dma_start(ft_st[:], featT[b, k * P : (k + 1) * P, :])
                    t = ftr_pool.tile([P, L], F32R, tag="ftr")
                    nc.vector.tensor_copy(t[:], ft_st[:])
                    ftr.append(t)
                def load_f():
                    f_r = []
                    for c in range(CL):
                        f_st = stage.tile([P, D], F32, tag="st", name=f"f_st{c}")
                        nc.sync.dma_start(f_st[:], feat[b, c * P : (c + 1) * P, :])
                        t = fr_pool.tile([P, D], F32R, tag="fr", name=f"f_r{c}")
                        nc.vector.tensor_copy(t[:], f_st[:])
                        f_r.append(t)
                    return f_r

                if not late_f:
                    f_r = load_f()

                score_ps = [psm.tile([1, NH], F32, tag="small", name=f"score_ps{h}") for h in range(2)]
                for cu in range(CU):
                    ps = pp.tile([P, L], F32, tag="proj")
                    for k in range(KD):
                        lhsT = w1r[:, k, cu * P : (cu + 1) * P]
                        nc.tensor.matmul(
                            ps[:, 0:NH], lhsT, ftr[k][:, 0:NH],
                            start=(k == 0), stop=(k == KD - 1),
                        )
                        nc.tensor.matmul(
                            ps[:, NH:L], lhsT, ftr[k][:, NH:L],
                            start=(k == 0), stop=(k == KD - 1),
                        )
                    th = tanh_pool.tile([P, L], F32R, tag="tanh")
                    nc.scalar.activation(
                        th[:], ps[:],
                        mybir.ActivationFunctionType.Tanh,
                        bias=biasT[:, cu * b_shard + b : cu * b_shard + b + 1],
                    )
                    for h in range(2):
                        nc.tensor.matmul(
                            score_ps[h][:, :],
                            v_r[:, cu : cu + 1],
                            th[:, h * NH : (h + 1) * NH],
                            start=(cu == 0),
                            stop=(cu == CU - 1),
                        )

                if late_f:
                    f_r = load_f()
                # softmax over l on one partition
                scr = row_pool.tile([1, L], F32, tag="scr")
                for h in range(2):
                    nc.vector.tensor_copy(scr[:, h * NH : (h + 1) * NH], score_ps[h][:, :])
                nmax = row_pool.tile([1, 1], F32, tag="nmax")
                nc.vector.reduce_max(nmax[:], scr[:], axis=mybir.AxisListType.X, negate=True)
                ex = row_pool.tile([1, L], F32, tag="ex")
                nc.scalar.activation(
                    ex[:], scr[:], mybir.ActivationFunctionType.Exp, bias=nmax[:]
                )
                sm = row_pool.tile([1, 1], F32, tag="sm")
                nc.vector.reduce_sum(sm[:], ex[:], axis=mybir.AxisListType.X)
                rcp = row_pool.tile([1, 1], F32, tag="rcp")
                nc.vector.reciprocal(rcp[:], sm[:])
                w_row = row_pool.tile([1, L], F32, tag="wrow")
                nc.vector.tensor_scalar_mul(w_row[:], ex[:], rcp[:])
                nc.sync.dma_start(attn_out[b : b + 1, :], w_row[:])

                # transpose w to partitions, round, context matmuls
                wt = row_pool.tile([P, CL], F32, tag="wt")
                w_scr = dram.tile([1, L], F32, tag="wscr")
                nc.sync.dma_start(w_scr[:], w_row[:])
                with nc.allow_non_contiguous_dma(reason="tiny per-batch w transpose"):
                    nc.sync.dma_start(
                        wt[:], w_scr[0].rearrange("(c l) -> l c", l=P)
                    )
                wt_r = row_pool.tile([P, CL], F32R, tag="wtr")
                nc.vector.tensor_copy(wt_r[:], wt[:])

                ctx_ps = [psm.tile([1, NH], F32, tag="small", name=f"ctx_ps{h}") for h in range(2)]
                for c in range(CL):
                    for h in range(2):
                        nc.tensor.matmul(
                            ctx_ps[h][:, :],
                            wt_r[:, c : c + 1],
                            f_r[c][:, h * NH : (h + 1) * NH],
                            start=(c == 0),
                            stop=(c == CL - 1),
                        )
                crow = row_pool.tile([1, D], F32, tag="crow")
                for h in range(2):
                    nc.vector.tensor_copy(crow[:, h * NH : (h + 1) * NH], ctx_ps[h][:, :])
                nc.sync.dma_start(ctx_out[b : b + 1, :], crow[:])
                ctx_row[b] = crow

    nc.finalize()
    return nc


_NC_CACHE = {}


def _get_nc(b_shard: int = B_SHARD):
    if b_shard not in _NC_CACHE:
        _NC_CACHE[b_shard] = build_nc(b_shard)
    return _NC_CACHE[b_shard]


def make_in_maps(features, hidden, W1, b1, W2, b2, V, bV):
    features = np.ascontiguousarray(np.asarray(features, dtype=np.float32))
    hidden = np.asarray(hidden, dtype=np.float32)
    W1 = np.ascontiguousarray(np.asarray(W1, dtype=np.float32))
    W2 = np.ascontiguousarray(np.asarray(W2, dtype=np.float32))
    V = np.ascontiguousarray(np.asarray(V, dtype=np.float32).reshape(U, 1))
    b1 = np.ascontiguousarray(np.asarray(b1, dtype=np.float32))
    b2 = np.ascontiguousarray(np.asarray(b2, dtype=np.float32))

    in_maps = []
    for i in range(N_CORES):
        sl = slice(i * B_SHARD, (i + 1) * B_SHARD)
        f = features[sl]
        in_maps.append(
            {
                "featT": np.ascontiguousarray(f.transpose(0, 2, 1)),
                "feat": np.ascontiguousarray(f),
                "hiddenT": np.ascontiguousarray(hidden[sl].T),
                "W1": W1,
                "W2": W2,
                "V": V,
                "b1": b1,
                "b2": b2,
            }
        )
    return in_maps


def kernel(features, hidden, W1, b1, W2, b2, V, bV):
    nc = _get_nc()
    in_maps = make_in_maps(features, hidden, W1, b1, W2, b2, V, bV)
    res = run_bass_kernel_spmd(nc, in_maps, list(range(N_CORES)))
    ctx = np.concatenate([res.results[i]["ctx"] for i in range(N_CORES)], axis=0)
    attn = np.concatenate([res.results[i]["attn"] for i in range(N_CORES)], axis=0)
    return ctx.astype(np.float32), attn.reshape(B, L, 1).astype(np.float32)
